# revision 66
# baseline (speedup 1.0000x reference)
"""Trainium2 Bass kernel for nn_DAFCN (motion-prediction DAFCN forward).

Structure exploited (verified vs the reference):
  * The attention branch (wq*/wk* convs, dvb) is dead code: the reference
    computes `combined[:, :, :DCT_N]` which selects only the GCN output.
  * The FFC branch (rfft -> 1x1 conv -> relu -> irfft, first 10 steps) is
    linear-relu-linear and is folded into matmuls (M1, M2, M3).
  * The iDCT + MLP are folded: h = relu(gcn_out @ A1 + ffc10 @ B1 + hb),
    out = h @ W2  with  A1 = (mlp_w1[:, :30] @ idct[:, :10]).T,
    B1 = mlp_w1[:, 30:40].T, W2 = mlp_w2[:10].T, and gc7_w folded into
    W7A = gc7_w @ A1, hb = gc7_b @ A1.
  * The DCT of the GCN input is folded into WG1F/A1 on the host
    (E^T @ W), so no on-device DCT matmul is needed.

Sharding: pure data parallelism - 1024 samples / 8 cores = 128 per core,
weights replicated.

fp8 GCN (HW-verified end-to-end rel err ~5.3e-3, gate 2e-2):
  * All gcb/gc7 matmuls run in fp8e4m3 with MatmulPerfMode.DoubleRow:
    each matmul contracts 2x128 K-rows at 0.5 cycles per output column
    (4x the bf16 FLOP rate under the cost model).
  * y state for a 16-sample group lives in ONE fp8 tile
    [128 part = d-in chunk, 4 kc, 2 group, 384 (s,n)] so DoubleRow lhsT
    slices [:, 2p:2p+2, g, j*128:+128] pair kc-chunks in a single AP.
  * Node mix: V = fp8 copy of u; chunks V0,V1 pair in one DoubleRow
    matmul over out cols 0:288 (start=True), V2 adds cols 240:384 with a
    plain fp8 matmul (start=False; "first write wins the zero" covers
    288:384, 240:288 accumulates the s5 straddle).
  * gc1 runs node-mix FIRST on a host-transposed x (xgt): its [10, 2,
    384] intermediate makes the PSUM->SBUF hop 768 free-columns instead
    of 3072; the K=10 feature mix (WG1T as lhsT) then lands each d-chunk
    directly in layout B.  (DoubleRow is NOT used there: a 10-wide
    stationary violates the dual-fp8 Ldweights ISA restrictions.)
  * Residual adds are folded into the next layer's feature-mix PSUM
    accumulation ((y+h)@W = y@W + h@W), removing them from the critical
    path; y1 = y0+h2 is materialized once on Pool for gc7.
  * zt PSUM tiles pair the two 8-sample groups: [128, 2, 512] f32 (two
    banks), so ONE tanh instruction (free size 768) covers both groups
    with a single per-partition bias (bias axis = d chunk = partitions).
  * The MLP (A1FB, W2) and FFC stay bf16: they carry the dominant
    dct_in residual, which is why fp8 noise in the GCN barely shows.

Scheduling: PSUM egress (tanh + V copies) is limited to ACT+DVE (Pool
and DMA cannot touch PSUM), so tanh owns ACT, V copies own DVE, and the
small FFC/gc7/output PSUM reads ride on ACT.  Emission is software-
pipelined: per-16-sample-group generators yield between small op chunks
and a round-robin scheduler co-runs 2-3 groups with a 2-stage skew, so
the 4-deep engine wait queues always see independent work.  gc7 keeps
its hp-tile section yield-free: interleaving another group's zt-ring
request between long-held hp slots creates a ring-release/PE-in-order
dependency cycle (scheduler deadlock).
"""

import numpy as np

import concourse.bass as bass
from concourse import mybir
from concourse.tile import TileContext

F32 = mybir.dt.float32
BF16 = mybir.dt.bfloat16
FP8 = mybir.dt.float8e4
AF = mybir.ActivationFunctionType
DR = mybir.MatmulPerfMode.DoubleRow

N_CORES = 8
B_TOT, T_IN, F_FEAT = 1024, 50, 48
SPC = B_TOT // N_CORES          # samples per core
D = 512
DCT_N = 10


# --------------------------------------------------------------------------
# host-side constant folding
# --------------------------------------------------------------------------

def _expand_att3(attT):
    """Node-mix rhs constants for the three 128-row V chunks of one
    8-sample group ((s, m) rows, sample-major, 48 nodes each).

    Returns (EA0, EA1, EA2): EA0/EA1 [128, 288] pair for the DoubleRow
    matmul over out cols 0:288, EA2 [128, 144] for out cols 240:384.
    """
    A = attT  # A[m, n] = att[n, m]
    EA0 = np.zeros((128, 288))
    EA0[0:48, 0:48] = A
    EA0[48:96, 48:96] = A
    EA0[96:128, 96:144] = A[0:32]
    EA1 = np.zeros((128, 288))
    EA1[0:16, 96:144] = A[32:48]
    EA1[16:64, 144:192] = A
    EA1[64:112, 192:240] = A
    EA1[112:128, 240:288] = A[0:16]
    EA2 = np.zeros((128, 144))
    EA2[0:32, 0:48] = A[16:48]
    EA2[32:80, 48:96] = A
    EA2[80:128, 96:144] = A
    return EA0, EA1, EA2


def _build_host_consts(inp):
    f8 = np.float64
    w1 = np.asarray(inp["mlp_w1"], f8)       # [256, 40]
    w2 = np.asarray(inp["mlp_w2"], f8)       # [40, 256]
    wg = np.asarray(inp["ffc_wg"], f8)       # [6, 6]
    wl = np.asarray(inp["ffc_wl"], f8)       # [3, 3]
    gc1_w = np.asarray(inp["gc1_w"], f8)     # [10, 512]
    gc1_b = np.asarray(inp["gc1_b"], f8)     # [512]
    gcb_w = np.asarray(inp["gcb_w"], f8)     # [2, 2, 512, 512]
    gcb_b = np.asarray(inp["gcb_b"], f8)     # [2, 2, 512]
    gc7_w = np.asarray(inp["gc7_w"], f8)     # [512, 10]
    gc7_b = np.asarray(inp["gc7_b"], f8)     # [10]
    att1 = np.asarray(inp["gc1_att"], f8)    # [48, 48]
    attb = np.asarray(inp["gcb_att"], f8)    # [2, 2, 48, 48]
    att7 = np.asarray(inp["gc7_att"], f8)    # [48, 48]

    # DCT pair (block length 30)
    N = 30
    kk = np.arange(N)[:, None]
    ii = np.arange(N)[None, :]
    w = np.full((N, 1), np.sqrt(2.0 / N))
    w[0, 0] = np.sqrt(1.0 / N)
    d = w * np.cos(np.pi * (ii + 0.5) * kk / N)
    idct = np.linalg.inv(d)
    dct10 = d[:DCT_N]                        # [10, 30]

    # E: dct_in^T[d, f] = sum_j E[d, j] * seq[40+j, f]
    E = dct10[:, :10].copy()
    E[:, 9] += dct10[:, 10:].sum(axis=1)
    ET = E.T                                 # [10 k, 10 d]

    # MLP folds
    A1 = (w1[:, :30] @ idct[:, :10]).T       # [10, 256]
    B1 = w1[:, 30:40].T                      # [10, 256]
    W7A = gc7_w @ A1                         # [512, 256]
    hb = gc7_b @ A1                          # [256]
    W2 = w2[:10].T                           # [256, 10]

    # FFC fold: rfft / channel mix / (relu) / irfft+local, first 10 steps
    Fm = np.fft.rfft(np.eye(60), axis=-1)    # [60, 31]
    Fr, Fi = Fm.real.T, Fm.imag.T            # [31, 60]
    M1 = (np.einsum("oc,kt->ctok", wg[:, :3], Fr)
          + np.einsum("oc,kt->ctok", wg[:, 3:], Fi)).reshape(3, 60, 186)
    M1f = np.concatenate(
        [M1[:, :49], M1[:, 49:].sum(axis=1, keepdims=True)], axis=1
    ).reshape(150, 186)                      # [(c,t<50), (o,k)]
    Gr = np.fft.irfft(np.eye(31), n=60, axis=-1)[:, :10]
    Gi = np.fft.irfft(1j * np.eye(31), n=60, axis=-1)[:, :10]
    M2 = np.zeros((6, 31, 3, 10))
    for o3 in range(3):
        M2[o3, :, o3, :] = Gr
        M2[o3 + 3, :, o3, :] = Gi
    M2 = M2.reshape(186, 30)                 # [(o,k), (o3,t')]
    M3 = np.einsum("oc,tu->ctou", wl, np.eye(10)).reshape(30, 30)

    import ml_dtypes
    c = {}
    f4 = lambda a: np.ascontiguousarray(a, ml_dtypes.bfloat16)
    q8 = lambda a: np.ascontiguousarray(a, ml_dtypes.float8_e4m3)

    # WGCB [128, 16, 512]: (layer l, k-chunk kc) -> w_l[kc*128+p, j]
    WGCB = np.zeros((128, 16, 512))
    for layer in range(4):
        s, ll = divmod(layer, 2)
        wl_ = gcb_w[s, ll]
        for kc in range(4):
            WGCB[:, layer * 4 + kc, :] = wl_[kc * 128:(kc + 1) * 128]
    c["WGCB"] = q8(WGCB)

    W7At = np.zeros((128, 4, 256))
    for kc in range(4):
        W7At[:, kc, :] = W7A[kc * 128:(kc + 1) * 128]
    c["W7AT"] = q8(W7At)

    # node-mix att constants, 6 layers (gc1, 4x gcb, gc7)
    atts = [att1, attb[0, 0], attb[0, 1], attb[1, 0], attb[1, 1], att7]
    ATTP = np.zeros((128, 6, 2, 288))
    ATT2 = np.zeros((128, 6, 144))
    for i, a in enumerate(atts):
        ea0, ea1, ea2 = _expand_att3(a.T)
        ATTP[:, i, 0, :] = ea0
        ATTP[:, i, 1, :] = ea1
        ATT2[:, i, :] = ea2
    c["ATTP"] = q8(ATTP)
    c["ATT2"] = q8(ATT2)

    # gc1 feature weights with the E fold (used as lhsT, K=10)
    c["WG1T"] = q8(ET @ gc1_w)               # [10, 512]

    # small bf16 constants packed into one tensor -> one startup DMA
    M2P = np.zeros((128, 2, 96))
    M3P = np.zeros((30, 96))
    for o3 in range(3):
        M2P[:, 0, o3 * 32:o3 * 32 + 10] = M2[:128].reshape(128, 3, 10)[:, o3]
        M2P[:58, 1, o3 * 32:o3 * 32 + 10] = M2[128:].reshape(58, 3, 10)[:, o3]
        M3P[:, o3 * 32:o3 * 32 + 10] = M3.reshape(30, 3, 10)[:, o3]
    BIAS = np.zeros((128, 22))
    tanh_biases = [gc1_b, gcb_b[0, 0], gcb_b[0, 1], gcb_b[1, 0], gcb_b[1, 1]]
    for li, b in enumerate(tanh_biases):
        for mc in range(4):
            BIAS[:, li * 4 + mc] = b[mc * 128:(mc + 1) * 128]
    for mc in range(2):
        BIAS[:, 20 + mc] = hb[mc * 128:(mc + 1) * 128]

    PACK = np.zeros((128, PACK_COLS))
    def put(name, arr):
        c0, c1, rows = PCOL[name]
        assert arr.shape == (rows, c1 - c0), (name, arr.shape)
        PACK[:rows, c0:c1] = arr
    tc_perm = np.array([c * 50 + t for t in range(50) for c in range(3)])
    M1TC = M1f[tc_perm]
    put("M1A", M1TC[:126])
    put("M1B", M1TC[126:150])
    put("BIAS", BIAS)
    put("M2P0", M2P[:, 0, :])
    put("M2P1", M2P[:58, 1, :])
    tc10 = np.array([c * 10 + t for t in range(10) for c in range(3)])
    put("M3P", M3P[tc10])
    # duplicate at base partition 32 to match X10's base partition
    c0, c1, rows = PCOL["M3P"]
    PACK[32:62, c0:c1] = M3P[tc10]
    A1FB = np.concatenate([ET @ A1, B1], axis=0)   # [20, 256]
    put("A1FB", A1FB)
    put("W2T0", W2[0:128, :])
    put("W2T1", W2[128:256, :])
    c["PACK"] = f4(PACK)
    c["HB32"] = np.ascontiguousarray(BIAS[:, 20:22], np.float32)
    return c


# packed-constant column map: name -> (col0, col1, rows)
PCOL = {
    "M1A": (0, 186, 126),
    "M1B": (186, 372, 24),
    "BIAS": (372, 394, 128),
    "M2P0": (394, 490, 128),
    "M2P1": (490, 586, 58),
    "M3P": (586, 682, 30),
    "A1FB": (682, 938, 20),
    "W2T0": (938, 948, 128),
    "W2T1": (948, 958, 128),
}
PACK_COLS = 958


CONST_SPECS = {
    "PACK": ((128, PACK_COLS), BF16),
    "HB32": ((128, 2), F32),
    "ATTP": ((128, 6, 2, 288), FP8),
    "ATT2": ((128, 6, 144), FP8),
    "WGCB": ((128, 16, 512), FP8),
    "W7AT": ((128, 4, 256), FP8),
    "WG1T": ((10, 512), FP8),
}


# --------------------------------------------------------------------------
# bass program
# --------------------------------------------------------------------------

def _split_matmul_waits(raw):
    """TRN2 walrus codegen allows only one sync-wait on Matmult/Ldweights.

    Move extra waits onto EventSemaphore instructions inserted just before
    (same engine, in-order execution => semantics preserved).
    """
    import json as _json
    bir = _json.loads(raw)
    for fn in bir["functions"]:
        for bb in fn["blocks"]:
            out = []
            for inst in bb["instructions"]:
                si = inst.get("sync_info")
                if (inst.get("opcode") != "EventSemaphore"
                        and si and len(si.get("on_wait") or []) > 1):
                    waits = si["on_wait"]
                    keep, extras = waits[-1], waits[:-1]
                    ip = len(out)
                    if (inst["opcode"] == "Matmult" and out
                            and out[-1].get("opcode") == "Ldweights"
                            and out[-1].get("engine") == inst["engine"]
                            and not (out[-1].get("sync_info") or {}).get(
                                "on_wait")):
                        ip = len(out) - 1
                    for j, w in enumerate(extras):
                        out.insert(ip + j, {
                            "debug": inst.get("debug", 0),
                            "engine": inst["engine"],
                            "ins": [], "outs": [],
                            "name": f"{inst['name']}_ws{j}",
                            "opcode": "EventSemaphore",
                            "sync_info": {"on_update": [], "on_wait": [w]},
                        })
                    si["on_wait"] = [keep]
                out.append(inst)
            bb["instructions"] = out
    return _json.dumps(bir).encode()


# engine assignment knobs (tuned via the cost model). PSUM can only be
# read by DVE and ACT (not Pool, not DMA), so all PSUM->SBUF egress is
# balanced across those two; Pool gets the SBUF->SBUF residual adds.
#   VCOPY_ENG[g][j]: engine for the (group, j-chunk) PSUM->SBUF V copy
#   RES_ENG: engines for the two residual-add halves
# per-(group, j) engine for the V copies
VCOPY_ENG = (("dve", "dve", "dve"), ("dve", "dve", "dve"))
V7_ENG = (("dve", "dve", "act"), ("dve", "dve", "dve"))
# tanh emitted per (mc, group) when True: doubles ACT instruction count
# but decouples the two 8-sample chains at every layer boundary
SPLIT_TANH = False
RES_ENG = ("pool", "pool")
HSB_ENG = ("act", "act")
ZSB_ENG = "act"
Z1_ENG = "dve"
PS_U_BUFS = 4
PS_ZT_BUFS = 2
FFCS_ENG = "act"
OSB_ENG = "act"
# admit the next sample group once the newest active one reaches this stage
SKEW_STAGE = 2
FIRST_SKEW = 2
LAST_SKEW = 2


def build_nc(spc=SPC):
    """Build the per-core Bass program for `spc` samples (multiple of 16)."""
    assert spc % 16 == 0
    n_sg = spc // 16
    nc = bass.Bass()

    xh = nc.declare_dram_parameter("xseq", [spc, T_IN, F_FEAT], BF16,
                                   isOutput=False)
    # gc1 input pre-transposed on the host: [(sg), chunk, (s,f) row, t]
    xg = nc.declare_dram_parameter("xgt", [spc // 16, 6, 128, 10], FP8,
                                   isOutput=False)
    # FFC inputs host-packed (t,c)-row-major with contiguous 512B runs:
    # rows 0:126 = XA (t 0:42), 126:150 = XB (t 42:50), 150:180 = X10
    xp = nc.declare_dram_parameter("xpk", [spc // 16, 188, 16, 16], BF16,
                                   isOutput=False)
    ch = {
        name: nc.declare_dram_parameter(name, list(shape), dt_, isOutput=False)
        for name, (shape, dt_) in CONST_SPECS.items()
    }
    oh = nc.declare_dram_parameter("out", [spc, DCT_N, 1, F_FEAT], F32,
                                   isOutput=True)

    with TileContext(nc) as tc:
        with (
            tc.tile_pool(name="consts", bufs=1) as consts,
            tc.tile_pool(name="seq", bufs=6) as p_seq,
            tc.tile_pool(name="zsb", bufs=6) as p_zsb,
            tc.tile_pool(name="ffc", bufs=6) as p_ffc,
            tc.tile_pool(name="vsb", bufs=8) as p_v,
            tc.tile_pool(name="ysb", bufs=6) as p_y,
            tc.tile_pool(name="hsb", bufs=8) as p_h,
            tc.tile_pool(name="hbig", bufs=6) as p_hbig,
            tc.tile_pool(name="osb", bufs=6) as p_osb,
            tc.tile_pool(name="ps_u", bufs=PS_U_BUFS, space="PSUM") as ps_u,
            tc.tile_pool(name="ps_zt", bufs=PS_ZT_BUFS, space="PSUM") as ps_zt,
        ):
            # ---- tiles + per-group input loader ----
            W = {
                name: consts.tile(list(shape), dt_, tag=name, name=name)
                for name, (shape, dt_) in CONST_SPECS.items()
            }

            def PK(name, cs=None):
                """Packed-constant AP: full rows, optional extra col slice."""
                c0, c1, rows = PCOL[name]
                if cs is not None:
                    c0, c1 = c0 + cs[0], min(c1, c0 + cs[1])
                return W["PACK"][0:rows, c0:c1]

            def load_inputs(g0):
                # 16 samples, (t,c)-major rows so each tile is one DMA:
                # XA rows (t 0:42, c), XB rows (t 42:50, c), X10 (t' 0:10, c)
                XA = p_seq.tile([126, 16, 16], BF16, tag="XA")
                XBT = p_seq.tile([62, 16, 16], BF16, tag="XBT")
                XB = XBT[0:24]
                X10 = XBT[32:62]  # base partition 32 (ISA requirement)
                # gc1 input, transposed layout: [(s,f) row, chunk, t]
                xT = p_seq.tile([128, 6, 10], FP8, tag="xT")
                nc.sync.dma_start(out=XA[...], in_=xp[g0 // 16, 0:126])
                nc.sync.dma_start(out=XBT[...], in_=xp[g0 // 16, 126:188])
                nc.sync.dma_start(
                    out=xT[...],
                    in_=xg[g0 // 16].rearrange("c p t -> p c t"),
                )
                # rows 40:50 bf16 stacked over the ffc result so the gc7
                # A1/B1 terms fold into one K=20 matmul
                s40b = p_seq.tile([20, 16, 48], BF16, tag="s40b")
                nc.sync.dma_start(
                    out=s40b[0:10, :, :],
                    in_=xh[g0:g0 + 16, 40:50].rearrange("b t f -> t b f"),
                )
                return XA, XB, X10, xT, s40b

            # ---- DMA issue order: PACK, sg0 inputs, then consts ----
            nc.sync.dma_start(out=W["PACK"][...], in_=ch["PACK"][...])
            inputs0 = load_inputs(0)
            nc.sync.dma_start(out=W["WG1T"][...], in_=ch["WG1T"][...])
            nc.sync.dma_start(out=W["ATTP"][:, 0:2], in_=ch["ATTP"][:, 0:2])
            nc.sync.dma_start(out=W["ATT2"][:, 0:2], in_=ch["ATT2"][:, 0:2])
            for l in range(2):
                nc.sync.dma_start(out=W["WGCB"][:, l * 4:(l + 1) * 4, :],
                                  in_=ch["WGCB"][:, l * 4:(l + 1) * 4, :])
            nc.sync.dma_start(out=W["ATTP"][:, 2:6], in_=ch["ATTP"][:, 2:6])
            nc.sync.dma_start(out=W["ATT2"][:, 2:6], in_=ch["ATT2"][:, 2:6])
            for l in range(2, 4):
                nc.sync.dma_start(out=W["WGCB"][:, l * 4:(l + 1) * 4, :],
                                  in_=ch["WGCB"][:, l * 4:(l + 1) * 4, :])
            nc.sync.dma_start(out=W["W7AT"][...], in_=ch["W7AT"][...])
            nc.sync.dma_start(out=W["HB32"][...], in_=ch["HB32"][...])

            def mm(out, lhsT, rhs, start=True, stop=True, perf_mode=None):
                nc.tensor.matmul(out=out, lhsT=lhsT, rhs=rhs,
                                 start=start, stop=stop, perf_mode=perf_mode)

            def vcopy(dst, src, eng):
                if eng == "dve":
                    nc.vector.tensor_copy(dst, src)
                else:
                    nc.scalar.copy(dst, src)

            def node_mix(zt_g, V01, V2, lidx, mc, start, stop):
                """zt_g [128, 384+] PSUM slice for one group: block-diag att
                mix of V chunks; DoubleRow pair (V0,V1) covers out cols
                0:288, plain fp8 V2 covers 240:384."""
                c0 = mc * 128
                mm(zt_g[:, 0:288], V01[:, :, c0:c0 + 128],
                   W["ATTP"][:, lidx, :, :], start=start, stop=False,
                   perf_mode=DR)
                mm(zt_g[:, 240:384], V2[:, c0:c0 + 128],
                   W["ATT2"][:, lidx, :], start=False, stop=stop)

            def gcn_layer(srcs, layer, out_pool, out_tag):
                """One GCN layer for one 16-sample group (both 8-groups).

                Generator (yields between op chunks so independent sample
                groups can interleave in issue order — the engine wait
                queues are only 4 deep, so long same-chain stretches
                head-of-line block everything behind them).

                srcs: a list of fp8 tiles [128, 4, 2, 384] whose
                feature-mix contributions SUM (this folds the residual add
                into the matmul accumulation: (y + h) @ W = y @ W + h @ W).
                Returns the tanh-output fp8 tile of the same shape (d-out
                chunk mc on dim 1).
                """
                # VV: [part, group, j-pair, d]; V2P: [part, group, d]
                VV = p_v.tile([128, 2, 2, 512], FP8, tag="vv", name="vv")
                V2P = p_v.tile([128, 2, 512], FP8, tag="v2", name="v2")
                for g in range(2):
                    for j in range(3):
                        u = ps_u.tile([128, 512], F32, tag="u", name="u")
                        n = len(srcs) * 2
                        for k, src_y in enumerate(srcs):
                            for p in range(2):
                                mm(u[...],
                                   src_y[:, 2 * p:2 * p + 2, g,
                                         j * 128:(j + 1) * 128],
                                   W["WGCB"][:, (layer - 1) * 4 + 2 * p:
                                             (layer - 1) * 4 + 2 * p + 2,
                                             :],
                                   start=(k == 0 and p == 0),
                                   stop=(k * 2 + p == n - 1),
                                   perf_mode=DR)
                        dst = VV[:, g, j, :] if j < 2 else V2P[:, g, :]
                        vcopy(dst, u[...], VCOPY_ENG[g][j])
                        yield
                o = out_pool.tile([128, 4, 2, 384], FP8, tag=out_tag,
                                  name=out_tag)
                for mc in range(4):
                    zt = ps_zt.tile([128, 2, 512], F32, tag="zt",
                                    name=f"zt{mc}")
                    for g in range(2):
                        node_mix(zt[:, g, :], VV[:, g, :, :], V2P[:, g, :],
                                 layer, mc, start=True, stop=True)
                    col = layer * 4 + mc
                    if SPLIT_TANH:
                        for g in range(2):
                            nc.scalar.activation(
                                o[:, mc, g, :], zt[:, g, 0:384], AF.Tanh,
                                bias=PK("BIAS", (col, col + 1)))
                    else:
                        nc.scalar.activation(o[:, mc, :, :], zt[:, :, 0:384],
                                             AF.Tanh,
                                             bias=PK("BIAS", (col, col + 1)))
                    yield
                return o

            def sg_stream(sg):
                """Full per-16-sample-group pipeline as a generator yielding
                its current stage index (0..6) at each chunk boundary."""
                g0 = sg * 16
                XA, XB, X10, xT, s40b = (inputs0 if sg == 0
                                         else load_inputs(g0))

                # ---- stage 0: FFC ----
                zp = ps_u.tile([128, 2, 256], F32, tag="u", name="zp")
                for m0, msz, sl in ((0, 128, 0), (128, 58, 1)):
                    mm(zp[0:msz, sl, :], PK("M1A", (m0, m0 + msz)),
                       XA[...], start=True, stop=False)
                    mm(zp[0:msz, sl, :], PK("M1B", (m0, m0 + msz)),
                       XB[...], start=False, stop=True)
                zsb = p_zsb.tile([128, 2, 256], BF16, tag="zsb")
                # relu via max(x, 0); rows 58:128 of slice 1 are junk but
                # never read downstream
                if ZSB_ENG == "act":
                    nc.scalar.activation(zsb[...], zp[...], AF.Relu)
                else:
                    nc.vector.tensor_scalar_max(zsb[...], zp[...], 0.0)
                yield 0
                fp = ps_u.tile([96, 256], F32, tag="u", name="fp")
                mm(fp[...], PK("M2P0"), zsb[:, 0, :],
                   start=True, stop=False)
                mm(fp[...], PK("M2P1"), zsb[0:58, 1, :],
                   start=False, stop=False)
                c0, c1, _ = PCOL["M3P"]
                mm(fp[...], W["PACK"][32:62, c0:c1], X10[...],
                   start=False, stop=True)
                ffc_s0 = p_ffc.tile([96, 16, 16], BF16, tag="ffc0")
                vcopy(ffc_s0[...], fp[...], FFCS_ENG)
                # partition-moving reshuffle (o3: partitions -> free) via
                # SBUF->SBUF DMA so the B1 matmul gets a contiguous out AP;
                # runs well before its gc7 consumer, latency fully hidden.
                for o3 in range(3):
                    nc.scalar.dma_start(
                        out=s40b[10:20, :, o3 * 16:(o3 + 1) * 16],
                        in_=ffc_s0[o3 * 32:o3 * 32 + 10, :, :])
                yield 0

                # ---- stages 1-5: gc1 + 2x2 gcb layers; residuals fold
                # into the next layer's feature accumulation (no add on the
                # critical path).  y1 = y0 + h2 is materialized on Pool off
                # the critical path, only for gc7's feature mix. ----
                def stage(gen, s):
                    while True:
                        try:
                            next(gen)
                        except StopIteration as e:
                            return e.value
                        yield s

                # gc1, node mix first: z1' = att-mix(x^T) is only [10, 384]
                # per group, so its PSUM->SBUF hop costs 768 free-columns
                # instead of 3072; the K=10 feature mix (WG1T as lhsT) then
                # lands each d-chunk directly in layout B for the tanh.
                z1p = ps_zt.tile([10, 2, 512], F32, tag="zt", name="z1p")
                for g in range(2):
                    # plain fp8 (DoubleRow with a 10-wide stationary violates
                    # the dual-fp8 Ldweights ISA restrictions)
                    mm(z1p[:, g, 0:288], xT[:, 3 * g, :],
                       W["ATTP"][:, 0, 0, :], start=True, stop=False)
                    mm(z1p[:, g, 0:288], xT[:, 3 * g + 1, :],
                       W["ATTP"][:, 0, 1, :], start=False, stop=False)
                    mm(z1p[:, g, 240:384], xT[:, 3 * g + 2, :],
                       W["ATT2"][:, 0, :], start=False, stop=True)
                z1s = p_zsb.tile([10, 2, 384], FP8, tag="z1s")
                vcopy(z1s[...], z1p[:, :, 0:384], Z1_ENG)
                yield 1
                y0 = p_y.tile([128, 4, 2, 384], FP8, tag="y8", name="y0")
                for mc in range(4):
                    zt = ps_zt.tile([128, 2, 512], F32, tag="zt",
                                    name=f"z1t{mc}")
                    for g in range(2):
                        mm(zt[:, g, 0:384],
                           W["WG1T"][:, mc * 128:(mc + 1) * 128],
                           z1s[:, g, :], start=True, stop=True)
                    nc.scalar.activation(y0[:, mc, :, :], zt[:, :, 0:384],
                                         AF.Tanh,
                                         bias=PK("BIAS", (mc, mc + 1)))
                    yield 1
                h1 = yield from stage(gcn_layer([y0], 1, p_h, "h8"), 2)
                h2 = yield from stage(gcn_layer([h1], 2, p_h, "h8"), 3)
                y1 = p_y.tile([128, 4, 2, 384], FP8, tag="y1", name="y1")
                for half in range(2):
                    sl = (slice(None), slice(2 * half, 2 * half + 2))
                    if RES_ENG[half] == "dve":
                        nc.vector.tensor_tensor(
                            out=y1[sl], in0=y0[sl], in1=h2[sl],
                            op=mybir.AluOpType.add)
                    else:
                        nc.gpsimd.tensor_add(y1[sl], y0[sl], h2[sl])
                h3 = yield from stage(gcn_layer([y0, h2], 3, p_h, "h8"), 4)
                h4 = yield from stage(gcn_layer([h3], 4, p_h, "h8"), 5)
                y = (y1, h4)  # gc7 feature mix sums these

                # ---- stage 6: gc7 + MLP.  NOTE: from the hp tile requests
                # through the last gc7 node matmul there must be NO yield:
                # the hp PSUM slots are held long, and letting another
                # stream's zt request slot in between creates a
                # ring-release / PE-in-order dependency cycle (deadlock).
                hps = []
                for mc in range(2):
                    hp = ps_zt.tile([128, 2, 512], F32, tag="zt",
                                    name=f"hp{mc}")
                    for g in range(2):
                        # (x @ A1 + ffc10 @ B1)^T in one K=20 matmul
                        mm(hp[:, g, 0:384],
                           PK("A1FB", (mc * 128, (mc + 1) * 128)),
                           s40b[:, g * 8:(g + 1) * 8, :],
                           start=True, stop=False)
                    hps.append(hp)
                VV7 = p_v.tile([128, 2, 2, 256], FP8, tag="vv7", name="vv7")
                V27 = p_v.tile([128, 2, 256], FP8, tag="v27", name="v27")
                for g in range(2):
                    for j in range(3):
                        u7 = ps_u.tile([128, 512], F32, tag="u", name="u7")
                        for k, src_y in enumerate(y):
                            for p in range(2):
                                mm(u7[:, 0:256],
                                   src_y[:, 2 * p:2 * p + 2, g,
                                         j * 128:(j + 1) * 128],
                                   W["W7AT"][:, 2 * p:2 * p + 2, :],
                                   start=(k == 0 and p == 0),
                                   stop=(k * 2 + p == 3), perf_mode=DR)
                        dst = (VV7[:, g, j, :] if j < 2
                               else V27[:, g, :])
                        vcopy(dst, u7[:, 0:256], V7_ENG[g][j])
                for g in range(2):
                    for mc in range(2):
                        c0 = mc * 128
                        mm(hps[mc][:, g, 0:288], VV7[:, g, :, c0:c0 + 128],
                           W["ATTP"][:, 5, :, :], start=False, stop=False,
                           perf_mode=DR)
                        mm(hps[mc][:, g, 240:384], V27[:, g, c0:c0 + 128],
                           W["ATT2"][:, 5, :], start=False, stop=True)
                yield 6
                # relu(h + hb), both groups per instruction
                hsbs = []
                for mc in range(2):
                    hsb = p_hbig.tile([128, 2, 384], BF16, tag="hbig")
                    args = dict(
                        out=hsb[...], in0=hps[mc][:, :, 0:384],
                        scalar1=W["HB32"][:, mc:mc + 1], scalar2=0.0,
                        op0=mybir.AluOpType.add, op1=mybir.AluOpType.max)
                    if HSB_ENG[mc] == "dve":
                        nc.vector.tensor_scalar(**args)
                    else:
                        nc.scalar.activation(
                            hsb[...], hps[mc][:, :, 0:384], AF.Relu,
                            bias=W["HB32"][:, mc:mc + 1])
                    hsbs.append(hsb)
                # out = (h @ W2)^T -> [10, (s,f)] per group (u-ring tiles:
                # short-lived, so their slot-release chains stay safe)
                osb = p_osb.tile([10, 2, 384], F32, tag="osb")
                for g in range(2):
                    op = ps_u.tile([10, 384], F32, tag="u", name="op")
                    for mc in range(2):
                        mm(op[...], PK("W2T1" if mc else "W2T0"),
                           hsbs[mc][:, g, :], start=(mc == 0), stop=(mc == 1))
                    vcopy(osb[:, g, :], op[...], OSB_ENG)
                nc.scalar.dma_start(
                    out=oh[g0:g0 + 16].rearrange("b t o f -> t b (o f)"),
                    in_=osb.rearrange("p g (s f) -> p (g s) f", f=F_FEAT),
                )
                yield 6

            # ---- software-pipelined emission: round-robin over 2-3
            # co-active sample groups so issue order interleaves
            # independent chains at chunk granularity ----
            active = [[0, sg_stream(0), -1]]
            next_sg = 1
            while active:
                for ent in list(active):
                    try:
                        ent[2] = next(ent[1])
                    except StopIteration:
                        active.remove(ent)
                thresh = (FIRST_SKEW if next_sg == 1
                          else LAST_SKEW if next_sg == n_sg - 1
                          else SKEW_STAGE)
                if (next_sg < n_sg and active
                        and active[-1][2] >= thresh):
                    active.append([next_sg, sg_stream(next_sg), -1])
                    next_sg += 1
    _orig_to_json_bytes = nc.to_json_bytes
    nc.to_json_bytes = lambda: _split_matmul_waits(_orig_to_json_bytes())
    return nc


# --------------------------------------------------------------------------
# host entry point
# --------------------------------------------------------------------------

_CACHE = {}


def kernel(**inputs):
    assert int(inputs.get("input_n", 50)) == 50
    assert int(inputs.get("output_n", 20)) == 20
    assert int(inputs.get("itera", 1)) == 1

    import ml_dtypes
    xf = np.asarray(inputs["input_seq"], np.float32)
    x = np.ascontiguousarray(xf.astype(ml_dtypes.bfloat16))
    # gc1 input, host-transposed: [group16, chunk, (s,f) row, t] fp8
    xgt = np.ascontiguousarray(
        xf[:, 40:50, :].reshape(B_TOT // 16, 16, 10, F_FEAT)
        .transpose(0, 1, 3, 2).reshape(B_TOT // 16, 6, 128, 10)
        .astype(ml_dtypes.float8_e4m3))
    # FFC inputs packed (t,c)-row-major: [group16, 180 rows, b16, f16]
    xr = (xf[:, 0:50, :].reshape(B_TOT // 16, 16, 50, 3, 16)
          .transpose(0, 2, 3, 1, 4).reshape(B_TOT // 16, 150, 16, 16))
    pad = np.zeros((B_TOT // 16, 8, 16, 16), np.float32)
    xpk = np.ascontiguousarray(
        np.concatenate([xr, pad, xr[:, 0:30]], axis=1)
        .astype(ml_dtypes.bfloat16))
    assert x.shape == (B_TOT, T_IN, F_FEAT)

    consts = _build_host_consts(inputs)

    if "nc" not in _CACHE:
        _CACHE["nc"] = build_nc(SPC)
    nc = _CACHE["nc"]

    from concourse.bass_utils import run_bass_kernel_spmd

    in_maps = []
    for i in range(N_CORES):
        m = dict(consts)
        m["xseq"] = x[i * SPC:(i + 1) * SPC]
        m["xgt"] = xgt[i * (SPC // 16):(i + 1) * (SPC // 16)]
        m["xpk"] = xpk[i * (SPC // 16):(i + 1) * (SPC // 16)]
        in_maps.append(m)

    res = run_bass_kernel_spmd(nc, in_maps, list(range(N_CORES)))
    out = np.concatenate([res.results[i]["out"] for i in range(N_CORES)],
                         axis=0)
    return out.astype(np.float32)
